# revision 2
# baseline (speedup 1.0000x reference)
"""Trainium2 Bass kernel for edge-softmax attention aggregation (GNN
message passing).

Host side: destination-sharded segment softmax (no cross-core collectives);
snake-deal nodes by degree into 8 cores x 50 blocks x 16 subblocks of 8
node slots (max 255 edges/subblock on this dataset -> c_sub=2, 2.4% pad);
cutoff/sqrt(dk) folded into q; q|k packed d-major, v h-major, all fp16.

Device (per core, SPMD), per pair of 128-node blocks:
  - SUB_NODES=8 / SUBS_PER_BLOCK=16 (snake-deal balances 8-node bins to
    max 255 edges on this dataset -> c_sub=2, same 2.4% padding): the
    one-hot is_equal shrinks 4x to [128, 8, 64] per pair.
  - Pair processing: each iteration handles the 2 blocks that arrive in
    one DMA transfer, halving per-op fixed overhead on DVE/ACT.
  - es*v is one merged op, software-pipelined one pair back so the DVE
    never stalls on ACT's exp.
  - All input DMA issued from the sync queue (HWDGE, no compute
    coupling); psum->SBUF copies run two pairs back on ACT.
  - Optional: first tree-add levels on GpSimd (TREE_ON_GPSIMD) - Q7
    supports fp ADD; watch for SBUF-port contention with DVE.
"""

import sys

if "/opt/trn_rl_repo" not in sys.path:
    sys.path.insert(0, "/opt/trn_rl_repo")

import numpy as np

import concourse.bacc as bacc
import concourse.mybir as mybir
import concourse.tile as tile
from concourse.bass_utils import run_bass_kernel_spmd

F32 = mybir.dt.float32
FP16 = mybir.dt.float16

N_NODES = 50000
N_EDGES = 1_600_000
DK = 64
H = 8
DH = 8
NC = 8

SUB_NODES = 8
SUBS_PER_BLOCK = 16
DEFAULT_BLOCKS = 50
GPB = SUBS_PER_BLOCK   # subblock groups per batch (one 128-node block)
OUT_GRP = 5            # pairs per output DMA

TREE_ON_GPSIMD = 0     # 0: tree on DVE; 1: t1 on gpsimd; 2: t1+t2 on gpsimd
PIPE = 3               # es*v / matmul software-pipeline depth (pairs)

DBG_MAP = {}           # inst name -> semantic label (filled at build)


def _dbg(op, label):
    try:
        DBG_MAP[op.ins.name] = label
    except Exception:
        pass
    return op


def build_program(c_sub: int, blocks: int, n_cores: int):
    g_core = blocks * SUBS_PER_BLOCK
    slots_sub = 128 * c_sub
    cs2 = GPB * c_sub                      # chunks per batch
    npairs = blocks // 2
    assert blocks % 2 == 0

    nc = bacc.Bacc("TRN2", target_bir_lowering=False, debug=False,
                   num_devices=n_cores)
    qkv = nc.declare_dram_parameter(
        "qkv", [g_core * slots_sub, 3 * DK], FP16, isOutput=False)
    lidx = nc.declare_dram_parameter(
        "lidx", [128, g_core * c_sub], FP16, isOutput=False)
    iota = nc.declare_dram_parameter(
        "iota", [128, SUB_NODES * 2 * cs2], FP16, isOutput=False)
    out = nc.declare_dram_parameter("out", [DK + 8, blocks * 128], FP16,
                                    isOutput=True)

    with tile.TileContext(nc) as tc, nc.allow_low_precision(
            "fp16 head-sum/es pipeline, ~7e-4 rel err vs fp32 reference"):
        with (
            tc.tile_pool(name="const", bufs=1) as cpool,
            tc.tile_pool(name="io", bufs=5) as iopool,
            tc.tile_pool(name="work", bufs=2) as wpool,
            tc.tile_pool(name="rwork", bufs=PIPE + 1) as rpool,
            tc.tile_pool(name="ohp", bufs=6) as ohpool,
            tc.tile_pool(name="psA", bufs=8, space="PSUM") as ppA,
            tc.tile_pool(name="outp", bufs=2) as opool,
        ):
            # iota_cs[p, j, ts] = j  (constant along the merged (t s) dim)
            iota_t = cpool.tile([128, SUB_NODES, 2 * cs2], FP16)
            nc.sync.dma_start(
                iota_t[:], iota[:].rearrange("p (j ts) -> p j ts",
                                             ts=2 * cs2))
            lidx_t = cpool.tile([128, g_core * c_sub], FP16)
            nc.sync.dma_start(lidx_t[:], lidx[:])
            nbias = cpool.tile([128, 1], F32)
            nc.vector.memset(nbias[:], -2.0)

            state = []   # pending pair payloads
            copyq = []   # pending (block_idx, psum, pair_idx)
            ob = None
            anchor = [None]   # most recent esr op (ACT ordering anchor)
            esr_of = {}       # pair -> esr op
            w_of = {}         # pair -> w op (DVE ordering pin)

            def flush_copies(upto):
                nonlocal ob
                while copyq and copyq[0][1] <= upto:
                    ps, pb = copyq.pop(0)
                    if pb % OUT_GRP == 0:
                        ob = opool.tile([DK + 8, OUT_GRP, 256], FP16)
                    cp = _dbg(nc.scalar.copy(
                        ob[:, pb % OUT_GRP, :],
                        ps[:].rearrange("p t n -> p (t n)")), f"copy.{pb}")
                    if anchor[0] is not None:
                        # pin the copy behind the newest exp in the ACT
                        # queue: the scheduler's PE model is optimistic and
                        # would otherwise hoist it into a 10us PE wait that
                        # stalls the in-order ACT stream
                        tile.add_dep_helper(
                            cp.ins, anchor[0].ins, sync=False,
                            reason="psum copy stays behind exp chain")
                    if pb % OUT_GRP == OUT_GRP - 1 or pb == npairs - 1:
                        p0g = (pb // OUT_GRP) * OUT_GRP
                        ng = pb - p0g + 1
                        nc.gpsimd.dma_start(
                            out[:, p0g * 256:(pb + 1) * 256],
                            ob[:, 0:ng, :].rearrange("p o d -> p (o d)"))

            def drain_pair():
                p0, dt0, rhs0, oh0, ps = state.pop(0)
                # merged es*v for pair p0, in place: rhs[0:64] holds the
                # replicated exp (written by ACT) and becomes es*v here
                ev = rhs0[:, :, :, 0:64].rearrange(
                    "p t s (h d) -> p t s h d", d=DH)
                esv_op = _dbg(nc.vector.tensor_tensor(
                    ev,
                    dt0[:, :, :, 128:192]
                    .rearrange("p t s (h d) -> p t s h d", d=DH),
                    ev,
                    op=mybir.AluOpType.mult), f"esv.{p0}")
                # order-only (same-engine) pin: esv(p0) sits after w(p0+2)
                # in the DVE stream, so it fills the DVE while ACT runs the
                # exp of pair p0+2; its own esr(p0) is 2 periods old -> no
                # semaphore wait. (A sync dep here would serialize; an
                # unpinned esv gets hoisted to zero distance by the
                # scheduler and ring-serializes at ~12us/pair.)
                dep = w_of.get(p0 + 2)
                if dep is not None:
                    tile.add_dep_helper(
                        esv_op.ins, dep.ins, sync=False,
                        reason="esv after w of pair+2 in DVE order")
                # scatter-add matmuls for both blocks of the pair
                for t in range(2):
                    for s in range(cs2):
                        j = s // c_sub
                        _dbg(nc.tensor.matmul(
                            ps[:, t, SUB_NODES * j:SUB_NODES * (j + 1)],
                            lhsT=rhs0[:, t, s, :],
                            rhs=oh0[:, :, t * cs2 + s],
                            start=(s % c_sub == 0),
                            stop=(s % c_sub == c_sub - 1)),
                            f"mm.{p0}.{t}.{s}")
                copyq.append((ps, p0))

            for p in range(npairs):
                dt = iopool.tile([128, 2, cs2, 3 * DK], FP16)
                if p == 0:
                    for t in range(2):
                        nc.sync.dma_start(
                            dt[:, t],
                            qkv[t * GPB * slots_sub:
                                (t + 1) * GPB * slots_sub, :]
                            .rearrange("(pp s) d -> pp s d", pp=128))
                else:
                    nc.sync.dma_start(
                        dt[:],
                        qkv[2 * p * GPB * slots_sub:
                            2 * (p + 1) * GPB * slots_sub, :]
                        .rearrange("(t pp s) d -> pp t s d", pp=128, t=2))

                # logits: qk mult + contiguous head-sum tree (all 2x fp16)
                qk = wpool.tile([128, 2 * cs2, DK], FP16)
                _dbg(nc.vector.tensor_tensor(
                    qk[:], dt[:].rearrange("p t s d -> p (t s) d")[:, :, 0:64],
                    dt[:].rearrange("p t s d -> p (t s) d")[:, :, 64:128],
                    op=mybir.AluOpType.mult), f"qk.{p}")
                t1 = wpool.tile([128, 2 * cs2, 32], FP16)
                t1_eng = nc.gpsimd if TREE_ON_GPSIMD >= 1 else nc.vector
                _dbg(t1_eng.tensor_tensor(
                    t1[:], qk[:, :, 0:32], qk[:, :, 32:64],
                    op=mybir.AluOpType.add), f"t1.{p}")
                t2 = wpool.tile([128, 2 * cs2, 16], FP16)
                t2_eng = nc.gpsimd if TREE_ON_GPSIMD >= 2 else nc.vector
                _dbg(t2_eng.tensor_tensor(
                    t2[:], t1[:, :, 0:16], t1[:, :, 16:32],
                    op=mybir.AluOpType.add), f"t2.{p}")
                w = wpool.tile([128, 2, cs2, H], FP16)
                w_of[p] = _dbg(nc.vector.tensor_tensor(
                    w[:].rearrange("p t s h -> p (t s) h"),
                    t2[:, :, 0:8], t2[:, :, 8:16],
                    op=mybir.AluOpType.add), f"w.{p}")

                # one-hot (8-wide) on DVE; 3d AP (merged t,s)
                oh = ohpool.tile([128, SUB_NODES, 2 * cs2], FP16)
                oh_op = _dbg(nc.vector.tensor_tensor(
                    oh[:],
                    lidx_t[:, 2 * p * cs2:2 * (p + 1) * cs2]
                    .rearrange("p (o ts) -> p o ts", o=1)
                    .to_broadcast([128, SUB_NODES, 2 * cs2]),
                    iota_t[:],
                    op=mybir.AluOpType.is_equal), f"oh.{p}")
                # order-only pin: keep oh(p) behind w(p) in the DVE stream;
                # unpinned, the scheduler hoists it to the head of a period
                # where its pool-slot WAR on the PE stalls the whole queue
                tile.add_dep_helper(
                    oh_op.ins, w_of[p].ins, sync=False,
                    reason="oh after w in DVE order")

                # rhs = [es*v | es]; denominator exp + replicated exp on ACT.
                # The replicated exp lands directly in rhs[0:64]; the es*v
                # multiply later runs in place (saves a whole esr tile).
                rhs = rpool.tile([128, 2, cs2, DK + 8], FP16)
                _dbg(nc.scalar.activation(rhs[:, :, :, 64:72], w[:],
                                          mybir.ActivationFunctionType.Exp,
                                          bias=nbias[:]), f"expden.{p}")
                anchor[0] = _dbg(nc.scalar.activation(
                    rhs[:, :, :, 0:64]
                    .rearrange("p t s (h d) -> p t s h d", d=DH),
                    w[:].rearrange("p t s (h o) -> p t s h o", o=1)
                    .to_broadcast([128, 2, cs2, H, DH]),
                    mybir.ActivationFunctionType.Exp, bias=nbias[:]),
                    f"esr.{p}")
                esr_of[p] = anchor[0]

                ps = ppA.tile([DK + 8, 2, 128], F32, name=f"ps{p}",
                              tag="psum")
                state.append((p, dt, rhs, oh, ps))

                # drain pair p-PIPE+1 after this pair's front ops: its esv
                # slots into the DVE queue right after w(p), overlapping
                # ACT's exp of pair p
                if len(state) >= PIPE:
                    drain_pair()

                # copies LAST in the iteration: they sit behind this pair's
                # exp ops in the in-order ACT queue and wait on PE matmuls
                # that completed a full period ago -> ACT never stalls on PE
                # (which would ring-serialize exp -> esv -> matmul -> copy)
                flush_copies(p - 4)

            while state:
                drain_pair()
            flush_copies(npairs)

    nc.compile()
    return nc


def prepare(key, value, query, edge_weight_cutoff, edge_index,
            blocks=DEFAULT_BLOCKS, n_cores=NC):
    """Host-side sharding (same scheme as kernel.py, finer subblocks)."""
    n_nodes = N_NODES
    n_edges = edge_index.shape[1]
    nsb = n_cores * blocks * SUBS_PER_BLOCK

    dst = np.asarray(edge_index[1], dtype=np.int64)
    deg = np.bincount(dst, minlength=n_nodes)

    order_nodes = np.argsort(-deg, kind="stable")
    rounds = -(-n_nodes // nsb)
    assert rounds <= SUB_NODES, "too few subblocks for node count"
    padded = np.full(rounds * nsb, -1, dtype=np.int64)
    padded[:n_nodes] = order_nodes
    arr = padded.reshape(rounds, nsb)
    arr[1::2] = arr[1::2, ::-1]
    bin_of_node = np.empty(n_nodes, dtype=np.int64)
    slot_of_node = np.empty(n_nodes, dtype=np.int64)
    rr, cc = np.divmod(np.arange(rounds * nsb), nsb)
    flat = arr.reshape(-1)
    mask = flat >= 0
    bin_of_node[flat[mask]] = cc[mask]
    slot_of_node[flat[mask]] = rr[mask]

    bin_edges = np.bincount(bin_of_node[dst], minlength=nsb)
    c_sub = max(1, int(-(-bin_edges.max() // 128)))
    slots_sub = 128 * c_sub

    sb_of_edge = bin_of_node[dst]
    eorder = np.argsort(sb_of_edge, kind="stable")
    counts = np.bincount(sb_of_edge, minlength=nsb)
    offsets = np.zeros(nsb + 1, dtype=np.int64)
    np.cumsum(counts, out=offsets[1:])
    sb_sorted = sb_of_edge[eorder]
    rank = np.arange(n_edges, dtype=np.int64) - offsets[sb_sorted]
    pp = rank // c_sub
    ss = rank % c_sub
    pos = ((sb_sorted // GPB) * (GPB * slots_sub) + pp * (GPB * c_sub)
           + (sb_sorted % GPB) * c_sub + ss)

    perm = np.full(nsb * slots_sub, n_edges, dtype=np.int64)
    perm[pos] = eorder
    lidx_flat = np.full(nsb * slots_sub, float(SUB_NODES + 7),
                        dtype=np.float16)
    lidx_flat[pos] = slot_of_node[dst[eorder]].astype(np.float16)

    scale = (np.asarray(edge_weight_cutoff, np.float32)
             * np.float32(1.0 / np.sqrt(DH)))
    dmaj = (np.arange(DK).reshape(H, DH).T.reshape(-1))
    packed = np.empty((n_edges + 1, 192), dtype=np.float16)
    packed[:n_edges, 0:64] = (np.asarray(query, np.float32)[:, dmaj]
                              * scale[:, None]).astype(np.float16)
    packed[:n_edges, 64:128] = np.asarray(key, np.float16)[:, dmaj]
    packed[:n_edges, 128:192] = np.asarray(value, np.float16)
    packed[n_edges] = 0.0

    g_core = blocks * SUBS_PER_BLOCK
    qkv_dev = packed[perm].reshape(n_cores, g_core * slots_sub, 192)
    lidx_dev = (lidx_flat.reshape(n_cores, g_core // GPB, 128, GPB * c_sub)
                .transpose(0, 2, 1, 3).reshape(n_cores, 128, g_core * c_sub))
    lidx_dev = np.ascontiguousarray(lidx_dev)
    cs2 = GPB * c_sub
    iota_np = np.tile(
        np.repeat(np.arange(SUB_NODES, dtype=np.float16), 2 * cs2), (128, 1))

    meta = dict(bin_of_node=bin_of_node, slot_of_node=slot_of_node, deg=deg,
                c_sub=c_sub, blocks=blocks, n_cores=n_cores)
    in_maps = [
        {"qkv": qkv_dev[c], "lidx": lidx_dev[c], "iota": iota_np}
        for c in range(n_cores)
    ]
    return in_maps, meta


def unshard(results, meta):
    n_cores = meta["n_cores"]
    blocks = meta["blocks"]
    g_core = blocks * SUBS_PER_BLOCK
    allout = []
    for c in range(n_cores):
        o = np.asarray(results[c]["out"]).astype(np.float32)
        num = o[0:64, :]
        den = o[64:72, :]
        with np.errstate(divide="ignore", invalid="ignore"):
            res = num / np.repeat(den, DH, axis=0)
        allout.append(res.T)
    allout = np.stack(allout)

    bin_of_node = meta["bin_of_node"]
    slot_of_node = meta["slot_of_node"]
    core = bin_of_node // g_core
    g = bin_of_node % g_core
    row = (g // SUBS_PER_BLOCK) * 128 \
        + (g % SUBS_PER_BLOCK) * SUB_NODES + slot_of_node
    out_full = allout[core, row]
    out_full[meta["deg"] == 0] = 0.0
    return out_full


_program_cache = {}


def kernel(key, value, query, edge_weight_cutoff, edge_index):
    in_maps, meta = prepare(key, value, query, edge_weight_cutoff, edge_index)
    cache_key = (meta["c_sub"], meta["blocks"], meta["n_cores"])
    if cache_key not in _program_cache:
        _program_cache[cache_key] = build_program(*cache_key)
    nc = _program_cache[cache_key]
    res = run_bass_kernel_spmd(nc, in_maps, list(range(meta["n_cores"])))
    return unshard(res.results, meta)


# revision 3
# speedup vs baseline: 1.1465x; 1.1465x over previous
"""Trainium2 Bass kernel for edge-softmax attention aggregation (GNN
message passing).

Host: destination-sharded segment softmax (no cross-core collectives);
snake-deal nodes by degree into 8 cores x 50 blocks x 16 subblocks of 8
node slots (max 255 edges/subblock on this dataset -> c_sub=2, 2.4% pad);
cutoff/sqrt(dk) folded into q; q|k packed d-major, v h-major, all fp16.

Device (per core, SPMD), one loop iteration per pair of 128-node blocks:
  - SUB_NODES=8 / SUBS_PER_BLOCK=16 (snake-deal balances 8-node bins to
    max 255 edges on this dataset -> c_sub=2, same 2.4% padding): the
    one-hot is_equal shrinks 4x to [128, 8, 64] per pair.
  - Pair processing: each iteration handles the 2 blocks that arrive in
    one DMA transfer, halving per-op fixed overhead on DVE/ACT.
  - es*v is one merged op, software-pipelined one pair back so the DVE
    never stalls on ACT's exp.
  - All input DMA issued from the sync queue (HWDGE, no compute
    coupling); psum->SBUF copies run two pairs back on ACT.
  - Optional: first tree-add levels on GpSimd (TREE_ON_GPSIMD) - Q7
    supports fp ADD; watch for SBUF-port contention with DVE.
"""

import sys

if "/opt/trn_rl_repo" not in sys.path:
    sys.path.insert(0, "/opt/trn_rl_repo")

import numpy as np

import concourse.bacc as bacc
import concourse.mybir as mybir
import concourse.tile as tile
from concourse.bass_utils import run_bass_kernel_spmd

F32 = mybir.dt.float32
FP16 = mybir.dt.float16

N_NODES = 50000
N_EDGES = 1_600_000
DK = 64
H = 8
DH = 8
NC = 8

SUB_NODES = 8
SUBS_PER_BLOCK = 16
DEFAULT_BLOCKS = 50
GPB = SUBS_PER_BLOCK   # subblock groups per batch (one 128-node block)
OUT_GRP = 5            # pairs per output DMA

TREE_ON_GPSIMD = 0     # 0: tree on DVE; 1: t1 on gpsimd; 2: t1+t2 on gpsimd
PIPE = 3               # es*v / matmul software-pipeline depth (pairs)

DBG_MAP = {}           # inst name -> semantic label (filled at build)


def _dbg(op, label):
    try:
        DBG_MAP[op.ins.name] = label
    except Exception:
        pass
    return op


def build_program(c_sub: int, blocks: int, n_cores: int):
    g_core = blocks * SUBS_PER_BLOCK
    slots_sub = 128 * c_sub
    cs2 = GPB * c_sub                      # chunks per batch
    npairs = blocks // 2
    assert blocks % 2 == 0

    nc = bacc.Bacc("TRN2", target_bir_lowering=False, debug=False,
                   num_devices=n_cores)
    qkv = nc.declare_dram_parameter(
        "qkv", [g_core * slots_sub, 3 * DK], FP16, isOutput=False)
    lidx = nc.declare_dram_parameter(
        "lidx", [128, g_core * c_sub], FP16, isOutput=False)
    iota = nc.declare_dram_parameter(
        "iota", [128, SUB_NODES * 2 * cs2], FP16, isOutput=False)
    out = nc.declare_dram_parameter("out", [DK + 8, blocks * 128], FP16,
                                    isOutput=True)

    with tile.TileContext(nc) as tc, nc.allow_low_precision(
            "fp16 head-sum/es pipeline, ~7e-4 rel err vs fp32 reference"):
        with (
            tc.tile_pool(name="const", bufs=1) as cpool,
            tc.tile_pool(name="io", bufs=5) as iopool,
            tc.tile_pool(name="work", bufs=2) as wpool,
            tc.tile_pool(name="rwork", bufs=PIPE + 1) as rpool,
            tc.tile_pool(name="ohp", bufs=6) as ohpool,
            tc.tile_pool(name="psA", bufs=8, space="PSUM") as ppA,
            tc.tile_pool(name="outp", bufs=2) as opool,
        ):
            nbias = cpool.tile([128, 1], F32)
            nc.vector.memset(nbias[:], -2.0)
            # pre-warm the exp table set during the first DMA wait (the
            # ACT_TABLE_LOAD costs ~2.7us; do it off the critical path)
            scratch = cpool.tile([128, 1], F32)
            nc.scalar.activation(scratch[:], nbias[:],
                                 mybir.ActivationFunctionType.Exp)
            iota_t = cpool.tile([128, SUB_NODES, 2 * cs2], FP16)
            lidx_t = cpool.tile([128, g_core * c_sub], FP16)

            state = []   # pending pair payloads
            copyq = []   # pending (block_idx, psum, pair_idx)
            ob = None
            anchor = [None]   # most recent esr op (ACT ordering anchor)
            esr_of = {}       # pair -> esr op
            w_of = {}         # pair -> w op (DVE ordering pin)

            def flush_copies(upto):
                nonlocal ob
                while copyq and copyq[0][1] <= upto:
                    ps, pb = copyq.pop(0)
                    if pb % OUT_GRP == 0:
                        ob = opool.tile([DK + 8, OUT_GRP, 256], FP16)
                    cp = _dbg(nc.scalar.copy(
                        ob[:, pb % OUT_GRP, :],
                        ps[:].rearrange("p t n -> p (t n)")), f"copy.{pb}")
                    if anchor[0] is not None:
                        # pin the copy behind the newest exp in the ACT
                        # queue: the scheduler's PE model is optimistic and
                        # would otherwise hoist it into a 10us PE wait that
                        # stalls the in-order ACT stream
                        tile.add_dep_helper(
                            cp.ins, anchor[0].ins, sync=False,
                            reason="psum copy stays behind exp chain")
                    if pb % OUT_GRP == OUT_GRP - 1 or pb == npairs - 1:
                        p0g = (pb // OUT_GRP) * OUT_GRP
                        ng = pb - p0g + 1
                        nc.gpsimd.dma_start(
                            out[:, p0g * 256:(pb + 1) * 256],
                            ob[:, 0:ng, :].rearrange("p o d -> p (o d)"))

            def drain_pair():
                p0, dt0, rhs0, oh0, ps = state.pop(0)
                # merged es*v for pair p0, in place: rhs[0:64] holds the
                # replicated exp (written by ACT) and becomes es*v here
                ev = rhs0[:, :, :, 0:64].rearrange(
                    "p t s (h d) -> p t s h d", d=DH)
                esv_op = _dbg(nc.vector.tensor_tensor(
                    ev,
                    dt0[:, :, :, 128:192]
                    .rearrange("p t s (h d) -> p t s h d", d=DH),
                    ev,
                    op=mybir.AluOpType.mult), f"esv.{p0}")
                # order-only (same-engine) pin: esv(p0) sits after w(p0+2)
                # in the DVE stream, so it fills the DVE while ACT runs the
                # exp of pair p0+2; its own esr(p0) is 2 periods old -> no
                # semaphore wait. (A sync dep here would serialize; an
                # unpinned esv gets hoisted to zero distance by the
                # scheduler and ring-serializes at ~12us/pair.)
                dep = w_of.get(p0 + 2)
                if dep is not None:
                    tile.add_dep_helper(
                        esv_op.ins, dep.ins, sync=False,
                        reason="esv after w of pair+2 in DVE order")
                # scatter-add matmuls for both blocks of the pair
                for t in range(2):
                    for s in range(cs2):
                        j = s // c_sub
                        _dbg(nc.tensor.matmul(
                            ps[:, t, SUB_NODES * j:SUB_NODES * (j + 1)],
                            lhsT=rhs0[:, t, s, :],
                            rhs=oh0[:, :, t * cs2 + s],
                            start=(s % c_sub == 0),
                            stop=(s % c_sub == c_sub - 1)),
                            f"mm.{p0}.{t}.{s}")
                copyq.append((ps, p0))

            for p in range(npairs):
                dt = iopool.tile([128, 2, cs2, 3 * DK], FP16)
                if p == 0:
                    # first pair lands before the constants: split halves so
                    # the pipeline starts as early as possible
                    for t in range(2):
                        nc.sync.dma_start(
                            dt[:, t],
                            qkv[t * GPB * slots_sub:
                                (t + 1) * GPB * slots_sub, :]
                            .rearrange("(pp s) d -> pp s d", pp=128))
                    nc.sync.dma_start(
                        iota_t[:], iota[:].rearrange("p (j ts) -> p j ts",
                                                     ts=2 * cs2))
                    nc.sync.dma_start(lidx_t[:], lidx[:])
                else:
                    nc.sync.dma_start(
                        dt[:],
                        qkv[2 * p * GPB * slots_sub:
                            2 * (p + 1) * GPB * slots_sub, :]
                        .rearrange("(t pp s) d -> pp t s d", pp=128, t=2))

                # logits: qk mult + contiguous head-sum tree (all 2x fp16)
                qk = wpool.tile([128, 2 * cs2, DK], FP16)
                _dbg(nc.vector.tensor_tensor(
                    qk[:], dt[:].rearrange("p t s d -> p (t s) d")[:, :, 0:64],
                    dt[:].rearrange("p t s d -> p (t s) d")[:, :, 64:128],
                    op=mybir.AluOpType.mult), f"qk.{p}")
                t1 = wpool.tile([128, 2 * cs2, 32], FP16)
                t1_eng = nc.gpsimd if TREE_ON_GPSIMD >= 1 else nc.vector
                _dbg(t1_eng.tensor_tensor(
                    t1[:], qk[:, :, 0:32], qk[:, :, 32:64],
                    op=mybir.AluOpType.add), f"t1.{p}")
                t2 = wpool.tile([128, 2 * cs2, 16], FP16)
                t2_eng = nc.gpsimd if TREE_ON_GPSIMD >= 2 else nc.vector
                _dbg(t2_eng.tensor_tensor(
                    t2[:], t1[:, :, 0:16], t1[:, :, 16:32],
                    op=mybir.AluOpType.add), f"t2.{p}")
                w = wpool.tile([128, 2, cs2, H], FP16)
                w_of[p] = _dbg(nc.vector.tensor_tensor(
                    w[:].rearrange("p t s h -> p (t s) h"),
                    t2[:, :, 0:8], t2[:, :, 8:16],
                    op=mybir.AluOpType.add), f"w.{p}")

                # one-hot (8-wide) on DVE; 3d AP (merged t,s)
                oh = ohpool.tile([128, SUB_NODES, 2 * cs2], FP16)
                oh_op = _dbg(nc.vector.tensor_tensor(
                    oh[:],
                    lidx_t[:, 2 * p * cs2:2 * (p + 1) * cs2]
                    .rearrange("p (o ts) -> p o ts", o=1)
                    .to_broadcast([128, SUB_NODES, 2 * cs2]),
                    iota_t[:],
                    op=mybir.AluOpType.is_equal), f"oh.{p}")
                # order-only pin: keep oh(p) behind w(p) in the DVE stream;
                # unpinned, the scheduler hoists it to the head of a period
                # where its pool-slot WAR on the PE stalls the whole queue
                tile.add_dep_helper(
                    oh_op.ins, w_of[p].ins, sync=False,
                    reason="oh after w in DVE order")

                # rhs = [es*v | es]; denominator exp + replicated exp on ACT.
                # The replicated exp lands directly in rhs[0:64]; the es*v
                # multiply later runs in place (saves a whole esr tile).
                rhs = rpool.tile([128, 2, cs2, DK + 8], FP16)
                _dbg(nc.scalar.activation(rhs[:, :, :, 64:72], w[:],
                                          mybir.ActivationFunctionType.Exp,
                                          bias=nbias[:]), f"expden.{p}")
                anchor[0] = _dbg(nc.scalar.activation(
                    rhs[:, :, :, 0:64]
                    .rearrange("p t s (h d) -> p t s h d", d=DH),
                    w[:].rearrange("p t s (h o) -> p t s h o", o=1)
                    .to_broadcast([128, 2, cs2, H, DH]),
                    mybir.ActivationFunctionType.Exp, bias=nbias[:]),
                    f"esr.{p}")
                esr_of[p] = anchor[0]

                ps = ppA.tile([DK + 8, 2, 128], F32, name=f"ps{p}",
                              tag="psum")
                state.append((p, dt, rhs, oh, ps))

                # drain pair p-PIPE+1 after this pair's front ops: its esv
                # slots into the DVE queue right after w(p), overlapping
                # ACT's exp of pair p
                if len(state) >= PIPE:
                    drain_pair()

                # copies LAST in the iteration: they sit behind this pair's
                # exp ops in the in-order ACT queue and wait on PE matmuls
                # that completed a full period ago -> ACT never stalls on PE
                # (which would ring-serialize exp -> esv -> matmul -> copy)
                flush_copies(p - 4)

            while state:
                drain_pair()
            flush_copies(npairs)

    nc.compile()
    return nc


def prepare(key, value, query, edge_weight_cutoff, edge_index,
            blocks=DEFAULT_BLOCKS, n_cores=NC):
    """Host-side sharding (same scheme as kernel.py, finer subblocks)."""
    n_nodes = N_NODES
    n_edges = edge_index.shape[1]
    nsb = n_cores * blocks * SUBS_PER_BLOCK

    dst = np.asarray(edge_index[1], dtype=np.int64)
    deg = np.bincount(dst, minlength=n_nodes)

    order_nodes = np.argsort(-deg, kind="stable")
    rounds = -(-n_nodes // nsb)
    assert rounds <= SUB_NODES, "too few subblocks for node count"
    padded = np.full(rounds * nsb, -1, dtype=np.int64)
    padded[:n_nodes] = order_nodes
    arr = padded.reshape(rounds, nsb)
    arr[1::2] = arr[1::2, ::-1]
    bin_of_node = np.empty(n_nodes, dtype=np.int64)
    slot_of_node = np.empty(n_nodes, dtype=np.int64)
    rr, cc = np.divmod(np.arange(rounds * nsb), nsb)
    flat = arr.reshape(-1)
    mask = flat >= 0
    bin_of_node[flat[mask]] = cc[mask]
    slot_of_node[flat[mask]] = rr[mask]

    bin_edges = np.bincount(bin_of_node[dst], minlength=nsb)
    c_sub = max(1, int(-(-bin_edges.max() // 128)))
    slots_sub = 128 * c_sub

    sb_of_edge = bin_of_node[dst]
    eorder = np.argsort(sb_of_edge, kind="stable")
    counts = np.bincount(sb_of_edge, minlength=nsb)
    offsets = np.zeros(nsb + 1, dtype=np.int64)
    np.cumsum(counts, out=offsets[1:])
    sb_sorted = sb_of_edge[eorder]
    rank = np.arange(n_edges, dtype=np.int64) - offsets[sb_sorted]
    pp = rank // c_sub
    ss = rank % c_sub
    pos = ((sb_sorted // GPB) * (GPB * slots_sub) + pp * (GPB * c_sub)
           + (sb_sorted % GPB) * c_sub + ss)

    perm = np.full(nsb * slots_sub, n_edges, dtype=np.int64)
    perm[pos] = eorder
    lidx_flat = np.full(nsb * slots_sub, float(SUB_NODES + 7),
                        dtype=np.float16)
    lidx_flat[pos] = slot_of_node[dst[eorder]].astype(np.float16)

    scale = (np.asarray(edge_weight_cutoff, np.float32)
             * np.float32(1.0 / np.sqrt(DH)))
    dmaj = (np.arange(DK).reshape(H, DH).T.reshape(-1))
    packed = np.empty((n_edges + 1, 192), dtype=np.float16)
    packed[:n_edges, 0:64] = (np.asarray(query, np.float32)[:, dmaj]
                              * scale[:, None]).astype(np.float16)
    packed[:n_edges, 64:128] = np.asarray(key, np.float16)[:, dmaj]
    packed[:n_edges, 128:192] = np.asarray(value, np.float16)
    packed[n_edges] = 0.0

    g_core = blocks * SUBS_PER_BLOCK
    qkv_dev = packed[perm].reshape(n_cores, g_core * slots_sub, 192)
    lidx_dev = (lidx_flat.reshape(n_cores, g_core // GPB, 128, GPB * c_sub)
                .transpose(0, 2, 1, 3).reshape(n_cores, 128, g_core * c_sub))
    lidx_dev = np.ascontiguousarray(lidx_dev)
    cs2 = GPB * c_sub
    iota_np = np.tile(
        np.repeat(np.arange(SUB_NODES, dtype=np.float16), 2 * cs2), (128, 1))

    meta = dict(bin_of_node=bin_of_node, slot_of_node=slot_of_node, deg=deg,
                c_sub=c_sub, blocks=blocks, n_cores=n_cores)
    in_maps = [
        {"qkv": qkv_dev[c], "lidx": lidx_dev[c], "iota": iota_np}
        for c in range(n_cores)
    ]
    return in_maps, meta


def unshard(results, meta):
    n_cores = meta["n_cores"]
    blocks = meta["blocks"]
    g_core = blocks * SUBS_PER_BLOCK
    allout = []
    for c in range(n_cores):
        o = np.asarray(results[c]["out"]).astype(np.float32)
        num = o[0:64, :]
        den = o[64:72, :]
        with np.errstate(divide="ignore", invalid="ignore"):
            res = num / np.repeat(den, DH, axis=0)
        allout.append(res.T)
    allout = np.stack(allout)

    bin_of_node = meta["bin_of_node"]
    slot_of_node = meta["slot_of_node"]
    core = bin_of_node // g_core
    g = bin_of_node % g_core
    row = (g // SUBS_PER_BLOCK) * 128 \
        + (g % SUBS_PER_BLOCK) * SUB_NODES + slot_of_node
    out_full = allout[core, row]
    out_full[meta["deg"] == 0] = 0.0
    return out_full


_program_cache = {}


def kernel(key, value, query, edge_weight_cutoff, edge_index):
    in_maps, meta = prepare(key, value, query, edge_weight_cutoff, edge_index)
    cache_key = (meta["c_sub"], meta["blocks"], meta["n_cores"])
    if cache_key not in _program_cache:
        _program_cache[cache_key] = build_program(*cache_key)
    nc = _program_cache[cache_key]
    res = run_bass_kernel_spmd(nc, in_maps, list(range(meta["n_cores"])))
    return unshard(res.results, meta)
